# revision 29
# baseline (speedup 1.0000x reference)
"""Trainium2 Bass kernel for nn_AffineExponential.

Computes, for each sample b:
    y_b   = expm(t_b * W) @ x_b + t_b * bias
    ljd_b = t_b * diag(W)

Key identity: expm(t W) x = sum_k (t^k / k!) W^k x, so instead of per-sample
matrix exponentials we run one shared chain of [128, B] matmuls with a scaled
recurrence  U_0 = X^T,  U_{k+1} = (W @ U_k) * t / (k+1)  and  y^T = sum_k U_k.
The per-column (per-sample) t scaling fuses into a single scalar_tensor_tensor
op per chain step on the vector engine; the running sum lives in SBUF with the
adds split between the vector and gpsimd engines (one batch-half each). K=10
terms reaches the fp32 floor (spectral radius of W ~1.08, t in [0,1); term k
magnitude <= 1.08^k/k!).

Sharding: pure data-parallel over the batch dim, 8 cores x 512 samples.
weight/bias replicated. All dims hardcoded per the harness contract.
"""

import sys
from contextlib import ExitStack

import numpy as np

for _p in ("/opt/trn_rl_repo", "/root/.axon_site/_ro/trn_rl_repo"):
    if _p not in sys.path:
        sys.path.append(_p)


def _ensure_ntff_hook_module():
    """The agent image's antenv lacks axon_hooks; provide it so
    run_bass_kernel_spmd's trace=True path can profile. No-op if present."""
    import types
    try:
        import antenv.axon_hooks  # noqa: F401
        return
    except ImportError:
        pass
    mod = types.ModuleType("antenv.axon_hooks")
    _state = {"hook": None}
    mod.set_axon_ntff_profile_hook = lambda h: _state.__setitem__("hook", h)
    mod.get_axon_ntff_profile_hook = lambda: _state["hook"]
    sys.modules["antenv.axon_hooks"] = mod
    try:
        from trn_agent_boot.trn_boot import _ntff_profile_via_ctypes
        mod.set_axon_ntff_profile_hook(
            _ntff_profile_via_ctypes("/opt/axon/libaxon_pjrt.so"))
    except Exception:
        pass


_ensure_ntff_hook_module()

import concourse.bass as bass
import concourse.tile as tile
from concourse import mybir
from concourse.bass_utils import run_bass_kernel_spmd

B, D = 4096, 128
N_CORES = 8
B_LOC = B // N_CORES  # 512
K_TERMS = 10  # terms 0..9; max-rel error 8.6e-7 vs fp32 reference
F32 = mybir.dt.float32
MULT = mybir.AluOpType.mult


def _hoist_waits(nc: bass.Bass) -> int:
    """Move semaphore waits off instructions onto standalone EventSemaphore
    instructions. This walrus build rejects any wait attached to a Matmult
    (S3_LW struct) and allows at most one elsewhere ("Too many sync wait
    commands"); a preceding same-engine wait instruction is equivalent."""
    n = 0
    for f in nc.m.functions:
        for blk in f.blocks:
            il = blk.instructions
            i = 0
            while i < len(il):
                ins = il[i]
                si = ins.sync_info
                if si is None or not si.on_wait:
                    i += 1
                    continue
                keep = 0 if ins.__class__.__name__ in ("InstMatmult", "InstMatmultMx") else 1
                waits = list(si.on_wait)
                if len(waits) <= keep:
                    i += 1
                    continue
                hoisted = waits[: len(waits) - keep]
                si.on_wait = waits[len(waits) - keep:]
                for w in hoisted:
                    wi = mybir.InstEventSemaphore(
                        name=f"W-hoist-{n}", engine=ins.engine, ins=[], outs=[])
                    wi.sync_info = type(si)(on_wait=[w], on_update=[])
                    il.insert(i, wi)
                    n += 1
                    i += 1
                i += 1
    return n


def _build_program(hoist: bool = True) -> bass.Bass:
    nc = bass.Bass("TRN2", target_bir_lowering=False, debug=False,
                   enable_asserts=False, num_devices=N_CORES,
                   enable_partition_id=False)

    # aux packs identity | ones | W so one DMA covers all [128, .] inputs;
    # tb packs t (as a row) | bias on partition 0.
    x_d = nc.dram_tensor("x", [B_LOC, D], F32, kind="ExternalInput").ap()
    aux_d = nc.dram_tensor("aux", [D, 3 * D], F32, kind="ExternalInput").ap()
    tb_d = nc.dram_tensor("tb", [1, B_LOC + D], F32, kind="ExternalInput").ap()
    y_d = nc.dram_tensor("y", [B_LOC, D], F32, kind="ExternalOutput").ap()
    ljd_d = nc.dram_tensor("ljd", [B_LOC, D], F32, kind="ExternalOutput").ap()

    NT = B_LOC // D  # 4 batch tiles of 128
    HALF = B_LOC // 2  # 256: chain runs as two independent column-halves

    with tile.TileContext(nc) as tc, ExitStack() as ctx:
        const = ctx.enter_context(tc.tile_pool(name="const", bufs=1))
        upool = ctx.enter_context(tc.tile_pool(name="u", bufs=6))
        ps_sm = ctx.enter_context(tc.tile_pool(name="ps_sm", bufs=3, space="PSUM"))
        ps_chain = ctx.enter_context(tc.tile_pool(name="ps_chain", bufs=3, space="PSUM"))
        ps_out = ctx.enter_context(tc.tile_pool(name="ps_out", bufs=2, space="PSUM"))

        # ---- PE pre-warm: the HAM clock gate keeps the PE at 1.2 GHz until
        # it sees a ~3.4us busy window. Dense dependency-free matmuls on
        # never-written scratch during the startup dead-time flip it to
        # 2.4 GHz before the real chain begins. ----
        scratch = const.tile([D, B_LOC], F32, tag="warm_scratch")
        nc.vector.memset(scratch, 0.0)
        for _ in range(3):
            psw = ps_chain.tile([D, B_LOC], F32, tag="ps_chain")
            nc.tensor.matmul(psw, scratch[:, 0:D], scratch)
        # early throwaway activation so the ACT table load overlaps startup
        warm_act = const.tile([1, 1], F32, tag="warm_act")
        nc.scalar.copy(warm_act, scratch[0:1, 0:1])

        # ---- loads (three DMAs, issued from different engines so the
        # per-queue trigger cost overlaps) ----
        x_bm = const.tile([D, NT, D], F32, tag="x_bm")
        nc.sync.dma_start(x_bm, x_d.rearrange("(m p) i -> p m i", p=D))
        aux = const.tile([D, 3 * D], F32, tag="aux")
        nc.scalar.dma_start(aux, aux_d)
        ident = aux[:, 0:D]
        ones_col = aux[:, D:D + 1]
        ones_row = aux[0:1, D:2 * D]
        w_sb = aux[:, 2 * D:3 * D]
        tb = const.tile([1, B_LOC + D], F32, tag="tb")
        nc.gpsimd.dma_start(tb, tb_d)
        t_row = tb[:, 0:B_LOC]
        bias_row = tb[:, B_LOC:B_LOC + D]

        # ---- layout transposes: XT = x^T (feature-major), WT = W^T ----
        xt = const.tile([D, B_LOC], F32, tag="xt")
        for m in range(NT):
            ps = ps_sm.tile([D, D], F32, tag="ps_sm")
            nc.tensor.transpose(ps, x_bm[:, m, :], ident)
            if m % 2 == 0:
                nc.scalar.copy(xt[:, bass.ts(m, D)], ps)
            else:
                nc.vector.tensor_copy(xt[:, bass.ts(m, D)], ps)
        wt = const.tile([D, D], F32, tag="wt")
        ps = ps_sm.tile([D, D], F32, tag="ps_sm")
        nc.tensor.transpose(ps, w_sb, ident)
        nc.vector.tensor_copy(wt, ps)

        # ---- T_rep[i, b] = t_b (broadcast across partitions, rank-1 matmul)
        t_rep = const.tile([D, B_LOC], F32, tag="t_rep")
        psT = ps_chain.tile([D, B_LOC], F32, tag="ps_chain")
        nc.tensor.matmul(psT, ones_row, t_row)
        nc.vector.tensor_copy(t_rep, psT)

        t2_rep = const.tile([D, B_LOC], F32, tag="t2_rep")
        nc.vector.tensor_mul(t2_rep, t_rep, t_rep)

        # ---- Taylor chain as two interleaved full-width chains over W^2
        # (even terms from U_0, odd terms from U_1): halves the serial
        # depth, one wide STT per term. fp32 matmuls cost two PE passes
        # (LOW/HIGH), so the running sum stays OFF the PE: two independent
        # SBUF accumulators (vector-owned / gpsimd-owned) merge once at the
        # end. diag/ljd matmuls slot into the first chain bubble. ----
        y_v = const.tile([D, B_LOC], F32, tag="y_v")
        y_g = const.tile([D, B_LOC], F32, tag="y_g")
        nc.gpsimd.tensor_copy(y_g, xt)   # term 0

        def chain_step(src, lhsT, scal, srep):
            psc = ps_chain.tile([D, B_LOC], F32, tag="ps_chain")
            nc.tensor.matmul(psc, lhsT, src)
            u = upool.tile([D, B_LOC], F32, tag="u")
            nc.vector.scalar_tensor_tensor(out=u, in0=psc, scalar=scal,
                                           in1=srep, op0=MULT, op1=MULT)
            return u[:]

        first_v = [True]

        def acc(u, k):
            if k in (1, 5, 7, 9):
                if first_v[0]:
                    nc.vector.tensor_copy(y_v, u)
                    first_v[0] = False
                else:
                    nc.vector.tensor_add(y_v, y_v, u)
            else:
                nc.gpsimd.tensor_add(y_g, y_g, u)

        odd = chain_step(xt, wt, 1.0, t_rep)          # U_1

        # W2T = (W^T)^2 for the dual chain (needed from the 2nd step on)
        psw2 = ps_sm.tile([D, D], F32, tag="ps_sm")
        nc.tensor.matmul(psw2, w_sb, wt)
        w2t = const.tile([D, D], F32, tag="w2t")
        nc.scalar.copy(w2t, psw2)

        even = xt
        pending = [(odd, 1)]
        assert K_TERMS == 10
        emitted_ljd = False
        for ke, ko in ((2, 3), (4, 5), (6, 7), (8, 9)):
            even = chain_step(even, w2t, float(1.0 / (ke * (ke - 1))), t2_rep)
            odd = chain_step(odd, w2t, float(1.0 / (ko * (ko - 1))), t2_rep)
            if not emitted_ljd:
                # diag(W) row + ljd = t x diag(W): independent PE work that
                # fills the bubble while the chain waits on the vector engine
                emitted_ljd = True
                wi = const.tile([D, D], F32, tag="wi")
                nc.gpsimd.tensor_mul(wi, w_sb, ident)
                psd = ps_sm.tile([D, D], F32, tag="ps_sm")
                nc.tensor.matmul(psd[0:1, :], ones_col, wi)
                diag_row = const.tile([1, D], F32, tag="diag_row")
                nc.scalar.copy(diag_row, psd[0:1, :])
                lo_all = const.tile([D, NT, D], F32, tag="lo_all")
                for m in range(NT):
                    psl = ps_out.tile([D, D], F32, tag="ps_out")
                    nc.tensor.matmul(psl, t_row[0:1, bass.ts(m, D)], diag_row)
                    nc.scalar.copy(lo_all[:, m, :], psl)
                nc.scalar.dma_start(
                    ljd_d.rearrange("(m p) i -> p m i", p=D), lo_all)
            for u, k in pending:
                acc(u, k)
            pending = [(even, ke), (odd, ko)]
        for u, k in pending:
            acc(u, k)
        y_fm = const.tile([D, B_LOC], F32, tag="y_fm")
        nc.vector.tensor_add(y_fm, y_v, y_g)

        # ---- transpose Y to batch-major, accumulating bias x t in PSUM ----
        yo_all = const.tile([D, NT, D], F32, tag="yo_all")
        for m in range(NT):
            ps = ps_out.tile([D, D], F32, tag="ps_out")
            nc.tensor.transpose(ps, y_fm[:, bass.ts(m, D)], ident)
            nc.tensor.matmul(ps, t_row[0:1, bass.ts(m, D)], bias_row,
                             start=False, stop=True, skip_group_check=True)
            if m % 2 == 0:
                nc.scalar.copy(yo_all[:, m, :], ps)
            else:
                nc.vector.tensor_copy(yo_all[:, m, :], ps)
        nc.sync.dma_start(y_d.rearrange("(m p) i -> p m i", p=D), yo_all)

    if hoist:
        _hoist_waits(nc)
    return nc


_CACHE: dict = {}


def _aux_np(w: np.ndarray) -> np.ndarray:
    c = np.zeros((D, 3 * D), dtype=np.float32)
    c[:, :D] = np.eye(D, dtype=np.float32)
    c[:, D:2 * D] = 1.0
    c[:, 2 * D:] = w
    return c


def _run(x, t, weight, bias, trace=False, **trace_kw):
    if "nc" not in _CACHE:
        _CACHE["nc"] = _build_program()
    nc = _CACHE["nc"]
    x = np.ascontiguousarray(x, dtype=np.float32)
    t = np.asarray(t, dtype=np.float32).reshape(B)
    w = np.asarray(weight, dtype=np.float32)
    bias = np.asarray(bias, dtype=np.float32).reshape(D)
    aux = _aux_np(w)
    in_maps = []
    for i in range(N_CORES):
        tb = np.concatenate([t[i * B_LOC:(i + 1) * B_LOC], bias])[None, :]
        in_maps.append({"x": x[i * B_LOC:(i + 1) * B_LOC],
                        "tb": np.ascontiguousarray(tb), "aux": aux})
    res = run_bass_kernel_spmd(nc, in_maps, list(range(N_CORES)),
                               trace=trace, **trace_kw)
    y = np.concatenate([res.results[i]["y"] for i in range(N_CORES)], axis=0)
    ljd = np.concatenate([res.results[i]["ljd"] for i in range(N_CORES)], axis=0)
    return (y, ljd), res


def kernel(x, t, weight, bias):
    (y, ljd), _ = _run(x, t, weight, bias, trace=False)
    return y, ljd


# revision 31
# speedup vs baseline: 1.2138x; 1.2138x over previous
"""Trainium2 Bass kernel for nn_AffineExponential.

Computes, for each sample b:
    y_b   = expm(t_b * W) @ x_b + t_b * bias
    ljd_b = t_b * diag(W)

Key identity: expm(t W) x = sum_k (t^k / k!) W^k x, so instead of per-sample
matrix exponentials we run one shared chain of [128, B] matmuls with a scaled
recurrence  U_0 = X^T,  U_{k+1} = (W @ U_k) * t / (k+1)  and  y^T = sum_k U_k.
The per-column (per-sample) t scaling fuses into a single scalar_tensor_tensor
op per chain step on the vector engine; the running sum lives in SBUF with the
adds split between the vector and gpsimd engines (one batch-half each). K=10
terms reaches the fp32 floor (spectral radius of W ~1.08, t in [0,1); term k
magnitude <= 1.08^k/k!).

Sharding: pure data-parallel over the batch dim, 8 cores x 512 samples.
weight/bias replicated. All dims hardcoded per the harness contract.
"""

import sys
from contextlib import ExitStack

import numpy as np

for _p in ("/opt/trn_rl_repo", "/root/.axon_site/_ro/trn_rl_repo"):
    if _p not in sys.path:
        sys.path.append(_p)


def _ensure_ntff_hook_module():
    """The agent image's antenv lacks axon_hooks; provide it so
    run_bass_kernel_spmd's trace=True path can profile. No-op if present."""
    import types
    try:
        import antenv.axon_hooks  # noqa: F401
        return
    except ImportError:
        pass
    mod = types.ModuleType("antenv.axon_hooks")
    _state = {"hook": None}
    mod.set_axon_ntff_profile_hook = lambda h: _state.__setitem__("hook", h)
    mod.get_axon_ntff_profile_hook = lambda: _state["hook"]
    sys.modules["antenv.axon_hooks"] = mod
    try:
        from trn_agent_boot.trn_boot import _ntff_profile_via_ctypes
        mod.set_axon_ntff_profile_hook(
            _ntff_profile_via_ctypes("/opt/axon/libaxon_pjrt.so"))
    except Exception:
        pass


_ensure_ntff_hook_module()

import concourse.bass as bass
import concourse.tile as tile
from concourse import mybir
from concourse.bass_utils import run_bass_kernel_spmd

B, D = 4096, 128
N_CORES = 8
B_LOC = B // N_CORES  # 512
K_TERMS = 10  # terms 0..9; max-rel error 8.6e-7 vs fp32 reference
F32 = mybir.dt.float32
MULT = mybir.AluOpType.mult


def _hoist_waits(nc: bass.Bass) -> int:
    """Move semaphore waits off instructions onto standalone EventSemaphore
    instructions. This walrus build rejects any wait attached to a Matmult
    (S3_LW struct) and allows at most one elsewhere ("Too many sync wait
    commands"); a preceding same-engine wait instruction is equivalent."""
    n = 0
    for f in nc.m.functions:
        for blk in f.blocks:
            il = blk.instructions
            i = 0
            while i < len(il):
                ins = il[i]
                si = ins.sync_info
                if si is None or not si.on_wait:
                    i += 1
                    continue
                keep = 0 if ins.__class__.__name__ in ("InstMatmult", "InstMatmultMx") else 1
                waits = list(si.on_wait)
                if len(waits) <= keep:
                    i += 1
                    continue
                hoisted = waits[: len(waits) - keep]
                si.on_wait = waits[len(waits) - keep:]
                for w in hoisted:
                    wi = mybir.InstEventSemaphore(
                        name=f"W-hoist-{n}", engine=ins.engine, ins=[], outs=[])
                    wi.sync_info = type(si)(on_wait=[w], on_update=[])
                    il.insert(i, wi)
                    n += 1
                    i += 1
                i += 1
    return n


def _trim_barriers(nc: bass.Bass) -> None:
    """Drop the preamble all-engine barrier (nothing reads the const-AP
    memsets it protects, and all semaphores start cleared) and the second
    tail barrier (engine queues simply end; the runtime re-dispatches only
    after every queue is exhausted). Keeps: the SP drain that guarantees
    output-DMA completion, barrier #1 that orders the semaphore clear after
    all work, and the EVSEM range clear that makes re-execution safe."""
    blocks = nc.m.functions[0].blocks
    main = blocks[0].instructions
    keep = [i for i in main if i.__class__.__name__ not in ("InstDrain", "InstEventSemaphore")]
    if len(keep) != len(main):
        del main[:]
        main.extend(keep)
    end = blocks[-1].instructions
    clear_idx = None
    for idx, ins in enumerate(end):
        if ins.__class__.__name__ == "InstEventSemaphoreRangeClear" or                 "RANGE_CLEAR" in str(getattr(ins, "opcode", "")) or                 "EVENT_SEMAPHORE_RANGE_CLEAR" in str(ins):
            clear_idx = idx
    if clear_idx is not None and clear_idx + 1 < len(end):
        del end[clear_idx + 1:]


def _build_program(hoist: bool = True) -> bass.Bass:
    nc = bass.Bass("TRN2", target_bir_lowering=False, debug=False,
                   enable_asserts=False, num_devices=N_CORES,
                   enable_partition_id=False)

    # aux packs identity | ones | W so one DMA covers all [128, .] inputs;
    # tb packs t (as a row) | bias on partition 0.
    x_d = nc.dram_tensor("x", [B_LOC, D], F32, kind="ExternalInput").ap()
    aux_d = nc.dram_tensor("aux", [D, 3 * D], F32, kind="ExternalInput").ap()
    tb_d = nc.dram_tensor("tb", [1, B_LOC + D], F32, kind="ExternalInput").ap()
    y_d = nc.dram_tensor("y", [B_LOC, D], F32, kind="ExternalOutput").ap()
    ljd_d = nc.dram_tensor("ljd", [B_LOC, D], F32, kind="ExternalOutput").ap()

    NT = B_LOC // D  # 4 batch tiles of 128
    HALF = B_LOC // 2  # 256: chain runs as two independent column-halves

    with tile.TileContext(nc) as tc, ExitStack() as ctx:
        const = ctx.enter_context(tc.tile_pool(name="const", bufs=1))
        upool = ctx.enter_context(tc.tile_pool(name="u", bufs=6))
        ps_sm = ctx.enter_context(tc.tile_pool(name="ps_sm", bufs=3, space="PSUM"))
        ps_chain = ctx.enter_context(tc.tile_pool(name="ps_chain", bufs=3, space="PSUM"))
        ps_out = ctx.enter_context(tc.tile_pool(name="ps_out", bufs=2, space="PSUM"))

        # ---- PE pre-warm: the HAM clock gate keeps the PE at 1.2 GHz until
        # it sees a ~3.4us busy window. Dense dependency-free matmuls on
        # never-written scratch during the startup dead-time flip it to
        # 2.4 GHz before the real chain begins. ----
        scratch = const.tile([D, B_LOC], F32, tag="warm_scratch")
        nc.vector.memset(scratch, 0.0)
        for _ in range(3):
            psw = ps_chain.tile([D, B_LOC], F32, tag="ps_chain")
            nc.tensor.matmul(psw, scratch[:, 0:D], scratch)
        # early throwaway activation so the ACT table load overlaps startup
        warm_act = const.tile([1, 1], F32, tag="warm_act")
        nc.scalar.copy(warm_act, scratch[0:1, 0:1])

        # ---- loads (three DMAs, issued from different engines so the
        # per-queue trigger cost overlaps) ----
        x_bm = const.tile([D, NT, D], F32, tag="x_bm")
        nc.sync.dma_start(x_bm, x_d.rearrange("(m p) i -> p m i", p=D))
        aux = const.tile([D, 3 * D], F32, tag="aux")
        nc.scalar.dma_start(aux, aux_d)
        ident = aux[:, 0:D]
        ones_col = aux[:, D:D + 1]
        ones_row = aux[0:1, D:2 * D]
        w_sb = aux[:, 2 * D:3 * D]
        tb = const.tile([1, B_LOC + D], F32, tag="tb")
        nc.gpsimd.dma_start(tb, tb_d)
        t_row = tb[:, 0:B_LOC]
        bias_row = tb[:, B_LOC:B_LOC + D]

        # ---- layout transposes: XT = x^T (feature-major), WT = W^T ----
        xt = const.tile([D, B_LOC], F32, tag="xt")
        for m in range(NT):
            ps = ps_sm.tile([D, D], F32, tag="ps_sm")
            nc.tensor.transpose(ps, x_bm[:, m, :], ident)
            nc.scalar.copy(xt[:, bass.ts(m, D)], ps)
        wt = const.tile([D, D], F32, tag="wt")
        ps = ps_sm.tile([D, D], F32, tag="ps_sm")
        nc.tensor.transpose(ps, w_sb, ident)
        nc.scalar.copy(wt, ps)

        # ---- T_rep[i, b] = t_b (broadcast across partitions, rank-1 matmul)
        t_rep = const.tile([D, B_LOC], F32, tag="t_rep")
        psT = ps_chain.tile([D, B_LOC], F32, tag="ps_chain")
        nc.tensor.matmul(psT, ones_row, t_row)
        nc.scalar.copy(t_rep, psT)

        t2_rep = const.tile([D, B_LOC], F32, tag="t2_rep")
        nc.vector.tensor_mul(t2_rep, t_rep, t_rep)

        # ---- Taylor chain as two interleaved full-width chains over W^2
        # (even terms from U_0, odd terms from U_1): halves the serial
        # depth, one wide STT per term. fp32 matmuls cost two PE passes
        # (LOW/HIGH), so the running sum stays OFF the PE: two independent
        # SBUF accumulators (vector-owned / gpsimd-owned) merge once at the
        # end. diag/ljd matmuls slot into the first chain bubble. ----
        y_v = const.tile([D, B_LOC], F32, tag="y_v")
        y_g = const.tile([D, B_LOC], F32, tag="y_g")
        nc.gpsimd.tensor_copy(y_g, xt)   # term 0

        def chain_step(src, lhsT, scal, srep):
            psc = ps_chain.tile([D, B_LOC], F32, tag="ps_chain")
            nc.tensor.matmul(psc, lhsT, src)
            u = upool.tile([D, B_LOC], F32, tag="u")
            nc.vector.scalar_tensor_tensor(out=u, in0=psc, scalar=scal,
                                           in1=srep, op0=MULT, op1=MULT)
            return u[:]

        first_v = [True]

        def acc(u, k):
            if k in (1, 5, 7, 9):
                if first_v[0]:
                    nc.vector.tensor_copy(y_v, u)
                    first_v[0] = False
                else:
                    nc.vector.tensor_add(y_v, y_v, u)
            else:
                nc.gpsimd.tensor_add(y_g, y_g, u)

        odd = chain_step(xt, wt, 1.0, t_rep)          # U_1

        # W2T = (W^T)^2 for the dual chain (needed from the 2nd step on)
        psw2 = ps_sm.tile([D, D], F32, tag="ps_sm")
        nc.tensor.matmul(psw2, w_sb, wt)
        w2t = const.tile([D, D], F32, tag="w2t")
        nc.scalar.copy(w2t, psw2)

        even = xt
        pending = [(odd, 1)]
        assert K_TERMS == 10
        emitted_ljd = False
        for ke, ko in ((2, 3), (4, 5), (6, 7), (8, 9)):
            even = chain_step(even, w2t, float(1.0 / (ke * (ke - 1))), t2_rep)
            odd = chain_step(odd, w2t, float(1.0 / (ko * (ko - 1))), t2_rep)
            if not emitted_ljd:
                # diag(W) row + ljd = t x diag(W): independent PE work that
                # fills the bubble while the chain waits on the vector engine
                emitted_ljd = True
                wi = const.tile([D, D], F32, tag="wi")
                nc.gpsimd.tensor_mul(wi, w_sb, ident)
                psd = ps_sm.tile([D, D], F32, tag="ps_sm")
                nc.tensor.matmul(psd[0:1, :], ones_col, wi)
                diag_row = const.tile([1, D], F32, tag="diag_row")
                nc.scalar.copy(diag_row, psd[0:1, :])
                lo_all = const.tile([D, NT, D], F32, tag="lo_all")
                for m in range(NT):
                    psl = ps_out.tile([D, D], F32, tag="ps_out")
                    nc.tensor.matmul(psl, t_row[0:1, bass.ts(m, D)], diag_row)
                    nc.scalar.copy(lo_all[:, m, :], psl)
                nc.scalar.dma_start(
                    ljd_d.rearrange("(m p) i -> p m i", p=D), lo_all)
            for u, k in pending:
                acc(u, k)
            pending = [(even, ke), (odd, ko)]
        for u, k in pending:
            acc(u, k)
        y_fm = const.tile([D, B_LOC], F32, tag="y_fm")
        nc.vector.tensor_add(y_fm, y_v, y_g)

        # ---- transpose Y to batch-major, accumulating bias x t in PSUM ----
        yo_all = const.tile([D, NT, D], F32, tag="yo_all")
        for m in range(NT):
            ps = ps_out.tile([D, D], F32, tag="ps_out")
            nc.tensor.transpose(ps, y_fm[:, bass.ts(m, D)], ident)
            nc.tensor.matmul(ps, t_row[0:1, bass.ts(m, D)], bias_row,
                             start=False, stop=True, skip_group_check=True)
            if m % 2 == 0:
                nc.scalar.copy(yo_all[:, m, :], ps)
            else:
                nc.vector.tensor_copy(yo_all[:, m, :], ps)
        nc.sync.dma_start(y_d.rearrange("(m p) i -> p m i", p=D), yo_all)

    _trim_barriers(nc)
    if hoist:
        _hoist_waits(nc)
    return nc


_CACHE: dict = {}


def _aux_np(w: np.ndarray) -> np.ndarray:
    c = np.zeros((D, 3 * D), dtype=np.float32)
    c[:, :D] = np.eye(D, dtype=np.float32)
    c[:, D:2 * D] = 1.0
    c[:, 2 * D:] = w
    return c


def _run(x, t, weight, bias, trace=False, **trace_kw):
    if "nc" not in _CACHE:
        _CACHE["nc"] = _build_program()
    nc = _CACHE["nc"]
    x = np.ascontiguousarray(x, dtype=np.float32)
    t = np.asarray(t, dtype=np.float32).reshape(B)
    w = np.asarray(weight, dtype=np.float32)
    bias = np.asarray(bias, dtype=np.float32).reshape(D)
    aux = _aux_np(w)
    in_maps = []
    for i in range(N_CORES):
        tb = np.concatenate([t[i * B_LOC:(i + 1) * B_LOC], bias])[None, :]
        in_maps.append({"x": x[i * B_LOC:(i + 1) * B_LOC],
                        "tb": np.ascontiguousarray(tb), "aux": aux})
    res = run_bass_kernel_spmd(nc, in_maps, list(range(N_CORES)),
                               trace=trace, **trace_kw)
    y = np.concatenate([res.results[i]["y"] for i in range(N_CORES)], axis=0)
    ljd = np.concatenate([res.results[i]["ljd"] for i in range(N_CORES)], axis=0)
    return (y, ljd), res


def kernel(x, t, weight, bias):
    (y, ljd), _ = _run(x, t, weight, bias, trace=False)
    return y, ljd


# revision 32
# speedup vs baseline: 1.2956x; 1.0674x over previous
"""Trainium2 Bass kernel for nn_AffineExponential.

Computes, for each sample b:
    y_b   = expm(t_b * W) @ x_b + t_b * bias
    ljd_b = t_b * diag(W)

Key identity: expm(t W) x = sum_k (t^k / k!) W^k x, so instead of per-sample
matrix exponentials we run one shared chain of [128, B] matmuls with a scaled
recurrence  U_0 = X^T,  U_{k+1} = (W @ U_k) * t / (k+1)  and  y^T = sum_k U_k.
The per-column (per-sample) t scaling fuses into a single scalar_tensor_tensor
op per chain step on the vector engine; the running sum lives in SBUF with the
adds split between the vector and gpsimd engines (one batch-half each). K=10
terms reaches the fp32 floor (spectral radius of W ~1.08, t in [0,1); term k
magnitude <= 1.08^k/k!).

Sharding: pure data-parallel over the batch dim, 8 cores x 512 samples.
weight/bias replicated. All dims hardcoded per the harness contract.
"""

import sys
from contextlib import ExitStack

import numpy as np

for _p in ("/opt/trn_rl_repo", "/root/.axon_site/_ro/trn_rl_repo"):
    if _p not in sys.path:
        sys.path.append(_p)


def _ensure_ntff_hook_module():
    """The agent image's antenv lacks axon_hooks; provide it so
    run_bass_kernel_spmd's trace=True path can profile. No-op if present."""
    import types
    try:
        import antenv.axon_hooks  # noqa: F401
        return
    except ImportError:
        pass
    mod = types.ModuleType("antenv.axon_hooks")
    _state = {"hook": None}
    mod.set_axon_ntff_profile_hook = lambda h: _state.__setitem__("hook", h)
    mod.get_axon_ntff_profile_hook = lambda: _state["hook"]
    sys.modules["antenv.axon_hooks"] = mod
    try:
        from trn_agent_boot.trn_boot import _ntff_profile_via_ctypes
        mod.set_axon_ntff_profile_hook(
            _ntff_profile_via_ctypes("/opt/axon/libaxon_pjrt.so"))
    except Exception:
        pass


_ensure_ntff_hook_module()

import concourse.bass as bass
import concourse.tile as tile
from concourse import mybir
from concourse.bass_utils import run_bass_kernel_spmd

B, D = 4096, 128
N_CORES = 8
B_LOC = B // N_CORES  # 512
K_TERMS = 10  # terms 0..9; max-rel error 8.6e-7 vs fp32 reference
F32 = mybir.dt.float32
MULT = mybir.AluOpType.mult


def _hoist_waits(nc: bass.Bass) -> int:
    """Move semaphore waits off instructions onto standalone EventSemaphore
    instructions. This walrus build rejects any wait attached to a Matmult
    (S3_LW struct) and allows at most one elsewhere ("Too many sync wait
    commands"); a preceding same-engine wait instruction is equivalent."""
    n = 0
    for f in nc.m.functions:
        for blk in f.blocks:
            il = blk.instructions
            i = 0
            while i < len(il):
                ins = il[i]
                si = ins.sync_info
                if si is None or not si.on_wait:
                    i += 1
                    continue
                keep = 0 if ins.__class__.__name__ in ("InstMatmult", "InstMatmultMx") else 1
                waits = list(si.on_wait)
                if len(waits) <= keep:
                    i += 1
                    continue
                hoisted = waits[: len(waits) - keep]
                si.on_wait = waits[len(waits) - keep:]
                for w in hoisted:
                    wi = mybir.InstEventSemaphore(
                        name=f"W-hoist-{n}", engine=ins.engine, ins=[], outs=[])
                    wi.sync_info = type(si)(on_wait=[w], on_update=[])
                    il.insert(i, wi)
                    n += 1
                    i += 1
                i += 1
    return n


def _trim_barriers(nc: bass.Bass) -> None:
    """Drop the preamble all-engine barrier (nothing reads the const-AP
    memsets it protects, and all semaphores start cleared) and the second
    tail barrier (engine queues simply end; the runtime re-dispatches only
    after every queue is exhausted). Keeps: the SP drain that guarantees
    output-DMA completion, barrier #1 that orders the semaphore clear after
    all work, and the EVSEM range clear that makes re-execution safe."""
    blocks = nc.m.functions[0].blocks
    main = blocks[0].instructions
    keep = [i for i in main if i.__class__.__name__ not in ("InstDrain", "InstEventSemaphore")]
    if len(keep) != len(main):
        del main[:]
        main.extend(keep)
    end = blocks[-1].instructions
    clear_idx = None
    for idx, ins in enumerate(end):
        if ins.__class__.__name__ == "InstEventSemaphoreRangeClear" or                 "RANGE_CLEAR" in str(getattr(ins, "opcode", "")) or                 "EVENT_SEMAPHORE_RANGE_CLEAR" in str(ins):
            clear_idx = idx
    if clear_idx is not None and clear_idx + 1 < len(end):
        del end[clear_idx + 1:]


def _build_program(hoist: bool = True) -> bass.Bass:
    nc = bass.Bass("TRN2", target_bir_lowering=False, debug=False,
                   enable_asserts=False, num_devices=N_CORES,
                   enable_partition_id=False)

    # aux packs identity | ones | W so one DMA covers all [128, .] inputs;
    # tb packs t (as a row) | bias on partition 0.
    x_d = nc.dram_tensor("x", [B_LOC, D], F32, kind="ExternalInput").ap()
    aux_d = nc.dram_tensor("aux", [D, 3 * D], F32, kind="ExternalInput").ap()
    tb_d = nc.dram_tensor("tb", [1, B_LOC + D], F32, kind="ExternalInput").ap()
    y_d = nc.dram_tensor("y", [B_LOC, D], F32, kind="ExternalOutput").ap()
    ljd_d = nc.dram_tensor("ljd", [B_LOC, D], F32, kind="ExternalOutput").ap()

    NT = B_LOC // D  # 4 batch tiles of 128
    HALF = B_LOC // 2  # 256: chain runs as two independent column-halves

    with tile.TileContext(nc) as tc, ExitStack() as ctx:
        const = ctx.enter_context(tc.tile_pool(name="const", bufs=1))
        upool = ctx.enter_context(tc.tile_pool(name="u", bufs=6))
        ps_sm = ctx.enter_context(tc.tile_pool(name="ps_sm", bufs=3, space="PSUM"))
        ps_chain = ctx.enter_context(tc.tile_pool(name="ps_chain", bufs=3, space="PSUM"))
        ps_out = ctx.enter_context(tc.tile_pool(name="ps_out", bufs=2, space="PSUM"))

        # ---- PE pre-warm: the HAM clock gate keeps the PE at 1.2 GHz until
        # it sees a ~3.4us busy window. Dense dependency-free matmuls on
        # never-written scratch during the startup dead-time flip it to
        # 2.4 GHz before the real chain begins. ----
        scratch = const.tile([D, B_LOC], F32, tag="warm_scratch")
        nc.vector.memset(scratch, 0.0)
        for _ in range(3):
            psw = ps_chain.tile([D, B_LOC], F32, tag="ps_chain")
            nc.tensor.matmul(psw, scratch[:, 0:D], scratch)
        # early throwaway activation so the ACT table load overlaps startup
        warm_act = const.tile([1, 1], F32, tag="warm_act")
        nc.scalar.copy(warm_act, scratch[0:1, 0:1])

        # ---- loads (three DMAs, issued from different engines so the
        # per-queue trigger cost overlaps) ----
        aux = const.tile([D, 3 * D], F32, tag="aux")
        nc.sync.dma_start(aux, aux_d)
        x_bm = const.tile([D, NT, D], F32, tag="x_bm")
        nc.sync.dma_start(x_bm, x_d.rearrange("(m p) i -> p m i", p=D))
        ident = aux[:, 0:D]
        ones_col = aux[:, D:D + 1]
        ones_row = aux[0:1, D:2 * D]
        w_sb = aux[:, 2 * D:3 * D]
        tb = const.tile([1, B_LOC + D], F32, tag="tb")
        nc.gpsimd.dma_start(tb, tb_d)
        t_row = tb[:, 0:B_LOC]
        bias_row = tb[:, B_LOC:B_LOC + D]

        # ---- layout transposes: XT = x^T (feature-major), WT = W^T ----
        xt = const.tile([D, B_LOC], F32, tag="xt")
        for m in range(NT):
            ps = ps_sm.tile([D, D], F32, tag="ps_sm")
            nc.tensor.transpose(ps, x_bm[:, m, :], ident)
            if m % 2 == 0:
                nc.scalar.copy(xt[:, bass.ts(m, D)], ps)
            else:
                nc.vector.tensor_copy(xt[:, bass.ts(m, D)], ps)
        wt = const.tile([D, D], F32, tag="wt")
        ps = ps_sm.tile([D, D], F32, tag="ps_sm")
        nc.tensor.transpose(ps, w_sb, ident)
        nc.scalar.copy(wt, ps)

        # ---- T_rep[i, b] = t_b (broadcast across partitions, rank-1 matmul)
        t_rep = const.tile([D, B_LOC], F32, tag="t_rep")
        psT = ps_chain.tile([D, B_LOC], F32, tag="ps_chain")
        nc.tensor.matmul(psT, ones_row, t_row)
        nc.scalar.copy(t_rep, psT)
        # keep the PE busy (HAM warm) while the setup copies drain
        for _ in range(2):
            psw = ps_chain.tile([D, B_LOC], F32, tag="ps_chain")
            nc.tensor.matmul(psw, scratch[:, 0:D], scratch)

        t2_rep = const.tile([D, B_LOC], F32, tag="t2_rep")
        nc.vector.tensor_mul(t2_rep, t_rep, t_rep)

        # ---- Taylor chain as two interleaved full-width chains over W^2
        # (even terms from U_0, odd terms from U_1): halves the serial
        # depth, one wide STT per term. fp32 matmuls cost two PE passes
        # (LOW/HIGH), so the running sum stays OFF the PE: two independent
        # SBUF accumulators (vector-owned / gpsimd-owned) merge once at the
        # end. diag/ljd matmuls slot into the first chain bubble. ----
        y_v = const.tile([D, B_LOC], F32, tag="y_v")
        y_g = const.tile([D, B_LOC], F32, tag="y_g")
        nc.gpsimd.tensor_copy(y_g, xt)   # term 0

        def chain_step(src, lhsT, scal, srep):
            psc = ps_chain.tile([D, B_LOC], F32, tag="ps_chain")
            nc.tensor.matmul(psc, lhsT, src)
            u = upool.tile([D, B_LOC], F32, tag="u")
            nc.vector.scalar_tensor_tensor(out=u, in0=psc, scalar=scal,
                                           in1=srep, op0=MULT, op1=MULT)
            return u[:]

        first_v = [True]

        def acc(u, k):
            if k in (1, 5, 7, 9):
                if first_v[0]:
                    nc.vector.tensor_copy(y_v, u)
                    first_v[0] = False
                else:
                    nc.vector.tensor_add(y_v, y_v, u)
            else:
                nc.gpsimd.tensor_add(y_g, y_g, u)

        odd = chain_step(xt, wt, 1.0, t_rep)          # U_1

        # W2T = (W^T)^2 for the dual chain (needed from the 2nd step on)
        psw2 = ps_sm.tile([D, D], F32, tag="ps_sm")
        nc.tensor.matmul(psw2, w_sb, wt)
        w2t = const.tile([D, D], F32, tag="w2t")
        nc.scalar.copy(w2t, psw2)

        even = xt
        pending = [(odd, 1)]
        assert K_TERMS == 10
        emitted_ljd = False
        for ke, ko in ((2, 3), (4, 5), (6, 7), (8, 9)):
            even = chain_step(even, w2t, float(1.0 / (ke * (ke - 1))), t2_rep)
            odd = chain_step(odd, w2t, float(1.0 / (ko * (ko - 1))), t2_rep)
            if not emitted_ljd:
                # diag(W) row + ljd = t x diag(W): independent PE work that
                # fills the bubble while the chain waits on the vector engine
                emitted_ljd = True
                wi = const.tile([D, D], F32, tag="wi")
                nc.gpsimd.tensor_mul(wi, w_sb, ident)
                psd = ps_sm.tile([D, D], F32, tag="ps_sm")
                nc.tensor.matmul(psd[0:1, :], ones_col, wi)
                diag_row = const.tile([1, D], F32, tag="diag_row")
                nc.scalar.copy(diag_row, psd[0:1, :])
                lo_all = const.tile([D, NT, D], F32, tag="lo_all")
                for m in range(NT):
                    psl = ps_out.tile([D, D], F32, tag="ps_out")
                    nc.tensor.matmul(psl, t_row[0:1, bass.ts(m, D)], diag_row)
                    nc.scalar.copy(lo_all[:, m, :], psl)
                nc.scalar.dma_start(
                    ljd_d.rearrange("(m p) i -> p m i", p=D), lo_all)
            for u, k in pending:
                acc(u, k)
            pending = [(even, ke), (odd, ko)]
        for u, k in pending:
            acc(u, k)
        y_fm = const.tile([D, B_LOC], F32, tag="y_fm")
        nc.vector.tensor_add(y_fm, y_v, y_g)

        # ---- transpose Y to batch-major, accumulating bias x t in PSUM ----
        yo_all = const.tile([D, NT, D], F32, tag="yo_all")
        for m in range(NT):
            ps = ps_out.tile([D, D], F32, tag="ps_out")
            nc.tensor.transpose(ps, y_fm[:, bass.ts(m, D)], ident)
            nc.tensor.matmul(ps, t_row[0:1, bass.ts(m, D)], bias_row,
                             start=False, stop=True, skip_group_check=True)
            if m % 2 == 0:
                nc.scalar.copy(yo_all[:, m, :], ps)
            else:
                nc.vector.tensor_copy(yo_all[:, m, :], ps)
        y_r = y_d.rearrange("(m p) i -> p m i", p=D)
        nc.sync.dma_start(y_r[:, 0:2, :], yo_all[:, 0:2, :])
        nc.sync.dma_start(y_r[:, 2:4, :], yo_all[:, 2:4, :])

    _trim_barriers(nc)
    if hoist:
        _hoist_waits(nc)
    return nc


_CACHE: dict = {}


def _aux_np(w: np.ndarray) -> np.ndarray:
    c = np.zeros((D, 3 * D), dtype=np.float32)
    c[:, :D] = np.eye(D, dtype=np.float32)
    c[:, D:2 * D] = 1.0
    c[:, 2 * D:] = w
    return c


def _run(x, t, weight, bias, trace=False, **trace_kw):
    if "nc" not in _CACHE:
        _CACHE["nc"] = _build_program()
    nc = _CACHE["nc"]
    x = np.ascontiguousarray(x, dtype=np.float32)
    t = np.asarray(t, dtype=np.float32).reshape(B)
    w = np.asarray(weight, dtype=np.float32)
    bias = np.asarray(bias, dtype=np.float32).reshape(D)
    aux = _aux_np(w)
    in_maps = []
    for i in range(N_CORES):
        tb = np.concatenate([t[i * B_LOC:(i + 1) * B_LOC], bias])[None, :]
        in_maps.append({"x": x[i * B_LOC:(i + 1) * B_LOC],
                        "tb": np.ascontiguousarray(tb), "aux": aux})
    res = run_bass_kernel_spmd(nc, in_maps, list(range(N_CORES)),
                               trace=trace, **trace_kw)
    y = np.concatenate([res.results[i]["y"] for i in range(N_CORES)], axis=0)
    ljd = np.concatenate([res.results[i]["ljd"] for i in range(N_CORES)], axis=0)
    return (y, ljd), res


def kernel(x, t, weight, bias):
    (y, ljd), _ = _run(x, t, weight, bias, trace=False)
    return y, ljd
